# revision 19
# baseline (speedup 1.0000x reference)
"""it-4a fallback: pair-based pass-2, device corr, finalize split.

Measured 207.9 us at full clock, rel err 1.748e-2.  Kept as a known-good
fallback for kernel.py.
"""

import functools

import numpy as np

B = 2
N = 8192
D = 256
N_CORES = 8
CORES_PER_BATCH = N_CORES // B
CHUNK = N // CORES_PER_BATCH
N_SUB = 8
N_PAIR = N_SUB // 2
N_REP = 4
SCALE = 1.0 / 16.0
VS = 8192.0


def _build_program(n=N, chunk=CHUNK, n_sub=N_SUB, n_devices=N_CORES,
                   enable_asserts=False):
    import concourse.bass as bass
    import concourse.tile as tile
    from concourse import bacc, mybir

    f32 = mybir.dt.float32
    f16 = mybir.dt.float16
    bf16 = mybir.dt.bfloat16
    fp8 = mybir.dt.float8e4
    ts = bass.ts
    P = 128
    DR = mybir.MatmulPerfMode.DoubleRow

    n_kt = chunk // P
    kq = n_kt // n_sub
    nqg = n // 1024
    nqb = n // 512

    nc = bacc.Bacc("TRN2", target_bir_lowering=False, debug=False,
                   enable_asserts=enable_asserts, num_devices=n_devices)

    xt8 = nc.dram_tensor("xt8", [N_REP, 2, P, n], fp8,
                         kind="ExternalInput").ap()
    xtb = nc.dram_tensor("xtb", [D, chunk], bf16, kind="ExternalInput").ap()
    a_t = nc.dram_tensor("a_t", [D, D], bf16, kind="ExternalInput").ap()
    wvt = nc.dram_tensor("wvt", [D, D], bf16, kind="ExternalInput").ap()
    out_part = nc.dram_tensor("out_part", [N_PAIR, 2, P, n], f16,
                              kind="ExternalOutput").ap()
    corr_out = nc.dram_tensor("corr_out", [P, 2], f32,
                              kind="ExternalOutput").ap()

    Exp = mybir.ActivationFunctionType.Exp

    with tile.TileContext(nc) as tc:
        with (
            tc.tile_pool(name="const", bufs=1) as const_pool,
            tc.tile_pool(name="proj", bufs=1) as proj_pool,
            tc.tile_pool(name="xq", bufs=1) as xq_pool,
            tc.tile_pool(name="xq8", bufs=1) as xq8_pool,
            tc.tile_pool(name="vpool", bufs=1) as v_pool,
            tc.tile_pool(name="gpool", bufs=1) as g_pool,
        ):
            ones8 = const_pool.tile([P, 1], fp8)
            nc.vector.memset(ones8[:], 1.0)

            A_sb = proj_pool.tile([P, 2, D], bf16)
            WvT_sb = proj_pool.tile([P, 2, D], bf16)
            V_sb = v_pool.tile([P, n_kt, D], bf16)
            G8 = [g_pool.tile([P, 2, chunk // 2], fp8, name=f"g8{kh}",
                              tag=f"g8{kh}") for kh in range(2)]
            xq_t = [xq_pool.tile([P, 2, 1024], bf16, name=f"xq{qc}",
                                 tag=f"xq{qc}") for qc in range(2)]
            xq8_t = [[xq8_pool.tile([P, 2, 1024], fp8, name=f"xq8_{r}_{qc}",
                                    tag=f"xq8_{r}_{qc}")
                      for qc in range(nqg)] for r in range(N_REP)]

            def xkey(kt, dh):
                return xq_t[kt // 8][:, dh, ts(kt % 8, P)]

            def g8s(ktg):
                return G8[ktg // 8][:, :, ts(ktg % 8, P)]

            with tc.tile_pool(name="psG", bufs=2, space="PSUM") as psG:
                nc.sync.dma_start(A_sb[:], a_t.rearrange("(c p) d -> p c d", p=P))
                for qc in range(2):
                    for dh in range(2):
                        nc.sync.dma_start(out=xq_t[qc][:, dh, :],
                                          in_=xtb[ts(dh, P), ts(qc, 1024)])
                nc.sync.dma_start(WvT_sb[:], wvt.rearrange("(c p) d -> p c d", p=P))
                # split replica loads across BOTH issue queues: the
                # lead-in is DMA-issue-bound (~621 ns/issue serial), and
                # sub 0 consumes r=0 (kt 0) and r=1 (kt 1) concurrently.
                for r in range(N_REP):
                    eng = nc.sync if r % 2 == 0 else nc.gpsimd
                    for qc in range(nqg):
                        for dh in range(2):
                            eng.dma_start(
                                out=xq8_t[r][qc][:, dh, :],
                                in_=xt8[r, dh, :, ts(qc, 1024)])

                for kh in range(2):
                    for dt in range(2):
                        gps = psG.tile([P, 1024], f32)
                        for dh in range(2):
                            for ks in range(2):
                                nc.tensor.matmul(
                                    gps[:, ts(ks, 512)],
                                    A_sb[:, dh, ts(dt, P)],
                                    xq_t[kh][:, dh, ts(ks, 512)],
                                    start=(dh == 0), stop=(dh == 1))
                        nc.vector.tensor_copy(G8[kh][:, dt, :], gps[:])

            with (
                tc.tile_pool(name="epool", bufs=4) as e_pool,
                tc.tile_pool(name="zpool", bufs=2) as z_pool,
                tc.tile_pool(name="vp", bufs=4) as vp_pool,
                tc.tile_pool(name="ostage", bufs=4) as o_pool,
                tc.tile_pool(name="psS", bufs=2, space="PSUM") as psS,
                tc.tile_pool(name="psO", bufs=3, space="PSUM") as psO,
                tc.tile_pool(name="psC", bufs=1, space="PSUM") as psC,
            ):
                E_gen = [None] * n_sub
                Vp_gen = [None] * n_sub
                cps = psC.tile([P, 4], f32)

                def v_unit(kt):
                    vps = psO.tile([P, 512], f32, tag="ops")
                    for dh in range(2):
                        nc.tensor.matmul(vps[:, :D], xkey(kt, dh),
                                         WvT_sb[:, dh, :],
                                         start=(dh == 0), stop=(dh == 1))
                    nc.vector.tensor_copy(V_sb[:, kt, :], vps[:, :D])

                ogrp = {"tile": None, "n": 0}

                def pass2_unit(pair, j, qb, copy_eng=None):
                    subs = (2 * pair, 2 * pair + 1)
                    ops = psO.tile([P, 512], f32, tag="ops")
                    for i, si in enumerate(subs):
                        nc.tensor.matmul(
                            ops[:], Vp_gen[si][:, :, ts(j, P)],
                            E_gen[si][:, :, ts(qb, 512)],
                            start=(i == 0), stop=(i == 1),
                            perf_mode=DR)
                    if ogrp["tile"] is None:
                        assert qb % 4 == 0
                        ogrp["tile"] = o_pool.tile([P, 4, 512], f16,
                                                   name="ostg", tag="ostg")
                        ogrp["n"] = 0
                    ost = ogrp["tile"]
                    sl = ogrp["n"]
                    assert sl == qb % 4
                    if copy_eng == "scalar":
                        nc.scalar.copy(ost[:, sl, :], ops[:])
                    else:
                        nc.vector.tensor_copy(ost[:, sl, :], ops[:])
                    ogrp["n"] += 1
                    if ogrp["n"] == 4:
                        nc.gpsimd.dma_start(
                            out_part[pair, j, :, ts(qb // 4, 2048)],
                            ost[:])
                        ogrp["tile"] = None

                fill = [("v", kt) for kt in range(n_kt)]
                state = {"rate": 1}

                def emit_fill(copy_eng=None):
                    for i in range(state["rate"]):
                        if not fill:
                            return
                        u = fill.pop(0)
                        if u[0] == "v":
                            v_unit(u[1])
                        else:
                            pass2_unit(*u[1:], copy_eng=copy_eng)

                def make_finalize(sub, Zp):
                    rzb_box = {}

                    def fin_dve():
                        Z = z_pool.tile([P, kq], f32)
                        nc.vector.tensor_reduce(
                            Z[:], Zp[:],
                            axis=mybir.AxisListType.X,
                            op=mybir.AluOpType.add)
                        Zs = z_pool.tile([P, kq], f32)
                        nc.vector.tensor_scalar_mul(Zs[:], Z[:], 1.0 / VS)
                        rz = z_pool.tile([P, kq], f32)
                        nc.vector.reciprocal(rz[:], Zs[:])
                        rzb = z_pool.tile([P, kq], bf16)
                        nc.vector.tensor_copy(rzb[:], rz[:])
                        rzb_box["rzb"] = rzb
                        Vp = vp_pool.tile([P, kq, D], fp8)
                        Vp_gen[sub] = Vp
                        for kt in range(kq):
                            nc.vector.tensor_scalar_mul(
                                Vp[:, kt, :], V_sb[:, sub * kq + kt, :],
                                rz[:, kt:kt + 1])

                        if sub % 2 == 1:
                            pair = sub // 2
                            fill.extend(("p2", pair, j, qb)
                                        for j in range(2)
                                        for qb in range(nqb))
                            state["rate"] = 2

                    def fin_pe():
                        rzb = rzb_box["rzb"]
                        Vp = Vp_gen[sub]
                        for kt in range(kq):
                            ktg = sub * kq + kt
                            sp = (ktg == n_kt - 1)
                            for j in range(2):
                                nc.tensor.matmul(
                                    cps[:, j:j + 1],
                                    V_sb[:, ktg, ts(j, P)],
                                    rzb[:, kt:kt + 1],
                                    start=(ktg == 0 and j == 0), stop=sp,
                                    skip_group_check=True)
                                nc.tensor.matmul(
                                    cps[:, 2 + j:3 + j],
                                    Vp[:, kt, ts(j, P)], ones8[:],
                                    start=False, stop=sp,
                                    skip_group_check=True)
                    return fin_dve, fin_pe

                pending_dve = pending_pe = None
                for sub in range(n_sub):
                    E_t = e_pool.tile([P, kq, n], fp8)
                    E_gen[sub] = E_t
                    Zp = z_pool.tile([P, kq, nqg], f32)

                    unit = 0
                    for kt in range(kq):
                        ktg = sub * kq + kt
                        for qg in range(nqg):
                            sps = psS.tile([P, 1024], f32)
                            for qb in range(2):
                                nc.tensor.matmul(
                                    sps[:, ts(qb, 512)],
                                    g8s(ktg),
                                    xq8_t[ktg % N_REP][qg][:, :, ts(qb, 512)],
                                    start=True, stop=True,
                                    perf_mode=DR)
                            nc.scalar.activation(
                                E_t[:, kt, ts(qg, 1024)], sps[:], Exp,
                                scale=SCALE,
                                accum_out=Zp[:, kt, qg:qg + 1])
                            if unit == 1 and pending_dve is not None:
                                pending_dve()
                                pending_dve = None
                            if unit == 9 and pending_pe is not None:
                                pending_pe()
                                pending_pe = None
                            if unit % 2 == 1:
                                emit_fill()
                            unit += 1
                    pending_dve, pending_pe = make_finalize(sub, Zp)

                pending_dve()
                pending_pe()

                c_sb = z_pool.tile([P, 4], f32)
                nc.vector.tensor_copy(c_sb[:], cps[:])
                corr = z_pool.tile([P, 2], f32)
                nc.vector.tensor_tensor(corr[:], c_sb[:, 0:2], c_sb[:, 2:4],
                                        mybir.AluOpType.subtract)
                nc.sync.dma_start(corr_out, corr[:])

                for i, u in enumerate(fill):
                    pass2_unit(*u[1:],
                               copy_eng="scalar" if i % 2 else "vector")

    nc.compile()
    return nc


@functools.lru_cache(maxsize=1)
def _get_compiled():
    return _build_program()


def _fp8_dither_reps(a):
    import ml_dtypes
    aa = np.maximum(np.abs(a), 2.0 ** -6)
    h = (2.0 ** np.floor(np.log2(aa))) / 8.0
    offs = (np.arange(N_REP) - (N_REP - 1) / 2) / N_REP
    return [(a + d * h).astype(ml_dtypes.float8_e4m3) for d in offs]


def kernel(x, Wq, Wk, Wv):
    import ml_dtypes
    from concourse.bass_utils import run_bass_kernel_spmd

    nc = _get_compiled()

    x = np.ascontiguousarray(x, dtype=np.float32)
    wq = np.asarray(Wq, dtype=np.float32)
    wk = np.asarray(Wk, dtype=np.float32)
    wv = np.asarray(Wv, dtype=np.float32)
    a_t = (wk.T @ wq).astype(ml_dtypes.bfloat16)
    wvt = np.ascontiguousarray(wv.T).astype(ml_dtypes.bfloat16)

    in_maps = []
    for c in range(N_CORES):
        b = c // CORES_PER_BATCH
        k0 = (c % CORES_PER_BATCH) * CHUNK
        xT = np.ascontiguousarray(np.roll(x[b].T, -k0, axis=1))
        reps = _fp8_dither_reps(xT)
        xt8 = np.stack([r.reshape(2, 128, N) for r in reps])
        in_maps.append({
            "xt8": xt8,
            "xtb": xT[:, :CHUNK].astype(ml_dtypes.bfloat16),
            "a_t": a_t,
            "wvt": wvt,
        })

    res = run_bass_kernel_spmd(nc, in_maps, list(range(N_CORES)))
    global LAST_RESULTS, LAST_EXEC_TIME_NS
    LAST_RESULTS = res
    LAST_EXEC_TIME_NS = res.exec_time_ns

    out = np.empty((B, N, D), dtype=np.float32)
    for b in range(B):
        acc = np.zeros((N, D), dtype=np.float32)
        for c in range(b * CORES_PER_BATCH, (b + 1) * CORES_PER_BATCH):
            k0 = (c % CORES_PER_BATCH) * CHUNK
            p = res.results[c]["out_part"].astype(np.float32)
            pT = p.sum(axis=0).reshape(D, N).T
            acc += np.roll(pT, k0, axis=0)
            corr = res.results[c]["corr_out"]
            acc += corr.T.reshape(D)[None, :]
        out[b] = acc * np.float32(1.0 / VS)
    return out
